# revision 4
# baseline (speedup 1.0000x reference)
"""DyDCNv2 (modulated deformable conv 3x3 + GroupNorm) on 8 Trainium2 cores. v4.

Every data-dependent access mechanism on TRN2 (SWDGE dma_gather ~180-440
ns/idx, ap_gather ~42 ns/idx, partition_broadcast ~190 us/call, DMA
partition-broadcast ~1.5 us/descriptor) is orders of magnitude too slow for
the 166K bilinear samples per core. v4 instead performs gather+blend as
dense PE matmuls:

  val[ch, s] = sum_j  X[row yy-7+j, :, ch]^T  @  S_j[:, s]
  S_j[cx, s] = tent(pyrel[s] - j) * tent(px8[s] - cx) * msk[s]

where tent(u) = relu(1 - |u|) is exactly the bilinear weight pair for both
corners at once, X is the image slab with 8 zero-padded columns on each
side (so out-of-image samples contribute zero like the reference's
zero-padding), and S_j is built in ONE custom DVE op per window row
(TENT_MUL_DCN: out = relu(s1 - |in0 - s0|) * in1, registered below).
The per-sample factors (pyrel, px8, msk) are computed with output rows on
partitions, then replicated across the 128 partitions by a K=1 ones-matmul
on the PE (the only full-rate replicator on this chip). The blended values
come out of PSUM in exactly the [cin, pix] layout the 9-tap conv matmuls
consume. GroupNorm statistics are reduced across the core pair sharing a
batch with a tiny AllReduce.

Sharding: core c handles batch b=c//2, row-half h=c%2 (48 of 96 rows).
"""

import os
import numpy as np
import ml_dtypes

import concourse.bass as bass
import concourse.bacc as bacc
import concourse.mybir as mybir
import concourse.tile as tile
import concourse.dve_ops as dve_ops
from concourse.dve_spec import Spec, lower as dve_lower, relu, maxx, Src0, Src1, C0, C1
from concourse.dve_uop import DveOpSpec
from concourse import library_config
from concourse.bass_utils import run_bass_kernel_spmd

P = 128
B, CIN, COUT, H, W = 4, 256, 256, 96, 96
HP = H // 2              # output rows per core
NPIX = HP * W            # 4608 output pixels per core
NT = 9                   # 3x3 taps
WIN = 14                 # source-row window per output row
SLAB = HP + WIN          # 62 slab rows resident per core
NS = NT * W              # 864 samples per output row (tap-major)
GN_GROUPS = 16
EPS = 1e-5
NG = (CIN // GN_GROUPS) * (H * W)  # elements per GN group (full image)

F32 = mybir.dt.float32
BF16 = mybir.dt.bfloat16
F16 = mybir.dt.float16
AX = mybir.AxisListType
OP = mybir.AluOpType

ITERS = int(os.environ.get("DCN_ITERS", "1"))
NOCC = os.environ.get("DCN_NOCC") == "1"
NOGMM = os.environ.get("DCN_NOGMM") == "1"
NOSJ = os.environ.get("DCN_NOSJ") == "1"
NOFREP = os.environ.get("DCN_NOFREP") == "1"
NOCONV = os.environ.get("DCN_NOCONV") == "1"

_CACHED = {}

_TENT_NAME = "TENT_MUL_DCN"


def _register_tent():
    """Register out = relu(s1 - |in0 - s0|) * in1 as a custom DVE op."""
    for op in dve_ops.OPS:
        if op.name == _TENT_NAME:
            return op
    row = dve_ops._CUSTOM_DVE_ROW_BASE + len(dve_ops.OPS)
    assert row < 0x20

    def _ref(in0, in1, s0, s1, imm2):
        a = np.maximum(in0.astype(np.float32) - s0, s0 - in0.astype(np.float32))
        return np.maximum(np.nan_to_num(s1 - a, nan=0.0), 0).astype(np.float32) * in1

    spec = Spec(
        body=relu(C1 - maxx(Src0 - C0, C0 - Src0)) * Src1,
        reference=_ref,
    )
    dve_ops._SUB_OPCODE_FOR_NAME[_TENT_NAME] = row
    shas = {}
    for ver in ("v3", "v4"):
        u = dve_lower(spec, ver=ver)
        shas[ver] = DveOpSpec(name=_TENT_NAME, opcode=row, uops=u, rd1_en=True).sha(ver)
    op = dve_ops.DveOp(_TENT_NAME, spec, subdim=False, uops_sha=shas)
    dve_ops.OPS.append(op)
    dve_ops.CUSTOM_DVE_SPECS[_TENT_NAME] = spec
    return op


TENT_OP = _register_tent()


def _build_nc():
    nc = bacc.Bacc("TRN2", target_bir_lowering=False, debug=False, num_devices=8)

    slabd = nc.dram_tensor("slabd", [P, SLAB * CIN], BF16, kind="ExternalInput")
    offs = nc.dram_tensor("offs", [18, NPIX], F32, kind="ExternalInput")
    mskd = nc.dram_tensor("mskd", [NT, NPIX], F32, kind="ExternalInput")
    wtd = nc.dram_tensor("wtd", [NT, CIN, COUT], BF16, kind="ExternalInput")
    dy7d = nc.dram_tensor("dy7d", [HP, NT, W], F32, kind="ExternalInput")
    bxi8d = nc.dram_tensor("bxi8d", [HP, NT, W], F32, kind="ExternalInput")
    seld = nc.dram_tensor("seld", [HP, HP * P], F32, kind="ExternalInput")
    iotapd = nc.dram_tensor("iotapd", [P, 1], F32, kind="ExternalInput")
    ind8d = nc.dram_tensor("ind8d", [P, 8], F32, kind="ExternalInput")
    e8d = nc.dram_tensor("e8d", [8, P], F32, kind="ExternalInput")
    gamd = nc.dram_tensor("gamd", [P, 2], F32, kind="ExternalInput")
    betd = nc.dram_tensor("betd", [P, 2], F32, kind="ExternalInput")
    yout = nc.dram_tensor("y", [COUT, NPIX], F32, kind="ExternalOutput")

    nc.gpsimd.load_library(library_config.mlp)

    with tile.TileContext(nc) as tc:
        with (
            tc.tile_pool(name="const", bufs=1) as cp,
            tc.tile_pool(name="dram", bufs=1, space="DRAM") as dp,
        ):
            # ---------------- true constants ----------------
            dy7 = cp.tile([HP, NT, W], F32, tag="dy7")
            nc.sync.dma_start(dy7[:], dy7d[:])
            bxi8 = cp.tile([HP, NT, W], F32, tag="bxi8")
            nc.sync.dma_start(bxi8[:], bxi8d[:])
            sel = cp.tile([HP, HP * P], F32, tag="sel")
            nc.sync.dma_start(sel[:], seld[:])
            iotap = cp.tile([P, 1], F32, tag="iotap")
            nc.sync.dma_start(iotap[:], iotapd[:])
            ind8 = cp.tile([P, 8], F32, tag="ind8")
            nc.sync.dma_start(ind8[:], ind8d[:])
            e8 = cp.tile([8, P], F32, tag="e8")
            nc.sync.dma_start(e8[:], e8d[:])
            gam = cp.tile([P, 2], F32, tag="gam")
            nc.sync.dma_start(gam[:], gamd[:])
            bet = cp.tile([P, 2], F32, tag="bet")
            nc.sync.dma_start(bet[:], betd[:])
            wtile = cp.tile([P, NT, 2, COUT], BF16, tag="wtile")
            nc.sync.dma_start(wtile[:], wtd.rearrange("t (kc p) o -> p t kc o", p=P))

            # persistent per-iter tensors
            slab = cp.tile([P, SLAB, CIN], BF16, tag="slab")
            cmp_ = cp.tile([HP, 3, NT, W], F32, tag="cmp")
            ysb = cp.tile([P, 2, NPIX], F32, tag="ysb")

            for _it in range(ITERS):
                nc.sync.dma_start(slab[:], slabd.rearrange("p (r c) -> p r c", c=CIN))

                # per-sample factors, output rows on partitions:
                # cmp[yy, 0, t, x] = pyrel = dy(t) + offy   (window-centered)
                # cmp[yy, 1, t, x] = px8   = x + dx(t) + offx + 8
                # cmp[yy, 2, t, x] = mask
                with tc.tile_pool(name="setup", bufs=1) as wk:
                    offw = offs.rearrange("(t two) (yy x) -> two yy t x", two=2, x=W)
                    offy = wk.tile([HP, NT, W], F32, tag="offy")
                    nc.sync.dma_start(offy[:], offw[0])
                    offx = wk.tile([HP, NT, W], F32, tag="offx")
                    nc.sync.dma_start(offx[:], offw[1])
                    nc.sync.dma_start(cmp_[:, 2],
                                      mskd.rearrange("t (yy x) -> yy t x", x=W))
                    nc.vector.tensor_tensor(cmp_[:, 0], offy[:], dy7[:], op=OP.add)
                    nc.vector.tensor_tensor(cmp_[:, 1], offx[:], bxi8[:], op=OP.add)

                # ---------------- main loop over output rows ----------------
                with (
                    tc.tile_pool(name="fr", bufs=3) as frp,
                    tc.tile_pool(name="bxp", bufs=3) as bxp,
                    tc.tile_pool(name="sj", bufs=6) as sjp,
                    tc.tile_pool(name="vs", bufs=3) as vsp,
                    tc.tile_pool(name="fp", bufs=1, space="PSUM") as fpp,
                    tc.tile_pool(name="val", bufs=1, space="PSUM") as valp,
                    tc.tile_pool(name="accp", bufs=1, space="PSUM") as accp,
                ):
                    for yy in range(HP):
                        # replicate this row's factors across all 128 partitions
                        fy16 = frp.tile([P, NS], F16, tag="fy16")
                        frep = frp.tile([P, 2, NS], F32, tag="frep")
                        if NOFREP:
                            nc.vector.memset(frep[:, :, 0:4], 0)
                            nc.vector.memset(fy16[:, 0:4], 0)
                        for f in (range(0) if NOFREP else range(3)):
                            fpsum = fpp.tile([P, 1024], F32, tag="fpsum",
                                             name="fpsum")
                            src = cmp_[:, f]
                            for lo, hi in ((0, 512), (512, NS)):
                                nc.tensor.matmul(
                                    fpsum[:, lo:hi],
                                    sel[:, yy * P:(yy + 1) * P],
                                    bass.AP(src.tensor, src.offset + lo,
                                            [src.ap[0], [1, hi - lo]]),
                                    start=True, stop=True)
                            if f == 0:
                                nc.scalar.copy(out=fy16[:], in_=fpsum[:, 0:NS])
                            else:
                                nc.scalar.copy(out=frep[:, f - 1, :],
                                               in_=fpsum[:, 0:NS])

                        # Bx[cx, s] = tent(px8[s] - cx) * msk[s]
                        bxm = bxp.tile([P, NS], F16, tag="bxm")
                        nc.vector._custom_dve(
                            TENT_OP, out=bxm[:], in0=frep[:, 0, :],
                            in1=frep[:, 1, :], s0=iotap[:], s1=1.0)

                        # gather matmuls: val[ch, s] accumulated over window rows
                        val = [valp.tile([P, 1024], F32, tag=f"val{kc}",
                                         name=f"val{kc}") for kc in range(2)]
                        for j in range(WIN):
                            sj = sjp.tile([P, NS], BF16, tag="sj")
                            if not NOSJ:
                                nc.vector._custom_dve(
                                    TENT_OP, out=sj[:], in0=fy16[:],
                                    in1=bxm[:], s0=float(j - 7), s1=1.0)
                            elif j == 0:
                                nc.vector.memset(sj[:], 0)
                            for kc in range(() if NOGMM else range(2)) if False else (range(0) if NOGMM else range(2)):
                                for lo, hi in ((0, 512), (512, NS)):
                                    nc.tensor.matmul(
                                        val[kc][:, lo:hi],
                                        slab[:, yy + j, kc * P:(kc + 1) * P],
                                        sj[:, lo:hi],
                                        start=(j == 0),
                                        stop=(j == WIN - 1),
                                    )

                        valS = vsp.tile([P, 2, NS], BF16, tag="valS")
                        if NOGMM:
                            if yy == 0:
                                nc.vector.memset(valS[:], 0)
                        else:
                            for kc in range(2):
                                nc.scalar.copy(out=valS[:, kc, :], in_=val[kc][:, 0:NS])

                        # conv: 9 taps x 2 cin-halves accumulated per cout-half
                        acc = [accp.tile([P, W], F32, tag=f"acc{cc}",
                                         name=f"acc{cc}") for cc in range(2)]
                        if not NOCONV:
                            for kc in range(2):
                                for cc in range(2):
                                    for t in range(NT):
                                        nc.tensor.matmul(
                                            acc[cc][:],
                                            wtile[:, t, kc, cc * P:(cc + 1) * P],
                                            valS[:, kc, t * W:(t + 1) * W],
                                            start=(kc == 0 and t == 0),
                                            stop=(kc == 1 and t == NT - 1),
                                        )
                        for cc in range(0 if NOCONV else 2):
                            nc.scalar.copy(out=ysb[:, cc, yy * W:(yy + 1) * W],
                                           in_=acc[cc][:])

                # ---------------- GroupNorm ----------------
                with tc.tile_pool(name="gnp", bufs=1, space="PSUM") as gnp, \
                     tc.tile_pool(name="gns", bufs=1) as wk:
                    st = wk.tile([P, 4], F32, tag="st")
                    sq = wk.tile([P, NPIX], F32, tag="sq")
                    for cc in range(2):
                        nc.vector.reduce_sum(st[:, 2 * cc:2 * cc + 1], ysb[:, cc, :], axis=AX.X)
                        nc.vector.tensor_tensor(sq[:], ysb[:, cc, :], ysb[:, cc, :], op=OP.mult)
                        nc.vector.reduce_sum(st[:, 2 * cc + 1:2 * cc + 2], sq[:], axis=AX.X)
                    pg = gnp.tile([8, 4], F32, tag="pg")
                    nc.tensor.matmul(pg[:], ind8[:], st[:], start=True, stop=True)
                    gsb = wk.tile([8, 4], F32, tag="gsb")
                    nc.vector.tensor_copy(gsb[:], pg[:])

                    cind = dp.tile([8, 4], F32, tag="cind")
                    cout_ = dp.tile([8, 4], F32, tag="cout")
                    nc.gpsimd.dma_start(cind[:], gsb[:])
                    if NOCC:
                        nc.sync.dma_start(cout_[:], cind[:])
                    else:
                        nc.gpsimd.collective_compute(
                            "AllReduce", OP.add,
                            replica_groups=[[0, 1], [2, 3], [4, 5], [6, 7]],
                            ins=[cind.opt()], outs=[cout_.opt()],
                        )
                    nc.sync.dma_start(gsb[:], cout_[:])

                    mu = wk.tile([8, 2], F32, tag="mu")
                    e2 = wk.tile([8, 2], F32, tag="e2")
                    nc.vector.tensor_scalar(mu[:], gsb[:, 0::2], 1.0 / NG, None, op0=OP.mult)
                    nc.vector.tensor_scalar(e2[:], gsb[:, 1::2], 1.0 / NG, None, op0=OP.mult)
                    m2t = wk.tile([8, 2], F32, tag="m2t")
                    nc.vector.tensor_tensor(m2t[:], mu[:], mu[:], op=OP.mult)
                    nc.vector.tensor_tensor(e2[:], e2[:], m2t[:], op=OP.subtract)
                    nc.vector.tensor_scalar(e2[:], e2[:], EPS, None, op0=OP.add)
                    rs = wk.tile([8, 2], F32, tag="rs")
                    nc.scalar.activation(rs[:], e2[:], mybir.ActivationFunctionType.Sqrt)
                    nc.vector.reciprocal(rs[:], rs[:])

                    pex = gnp.tile([P, 2], F32, tag="pex")
                    rsc = wk.tile([P, 2], F32, tag="rsc")
                    nc.tensor.matmul(pex[:], e8[:], rs[:], start=True, stop=True)
                    nc.vector.tensor_copy(rsc[:], pex[:])
                    pex2 = gnp.tile([P, 2], F32, tag="pex2")
                    muc = wk.tile([P, 2], F32, tag="muc")
                    nc.tensor.matmul(pex2[:], e8[:], mu[:], start=True, stop=True)
                    nc.vector.tensor_copy(muc[:], pex2[:])

                    sc = wk.tile([P, 2], F32, tag="sc")
                    nc.vector.tensor_tensor(sc[:], rsc[:], gam[:], op=OP.mult)
                    sh = wk.tile([P, 2], F32, tag="sh")
                    nc.vector.tensor_tensor(sh[:], muc[:], sc[:], op=OP.mult)
                    nc.vector.tensor_tensor(sh[:], bet[:], sh[:], op=OP.subtract)

                    for cc in range(2):
                        nc.vector.tensor_scalar(
                            ysb[:, cc, :], ysb[:, cc, :],
                            sc[:, cc:cc + 1], sh[:, cc:cc + 1],
                            op0=OP.mult, op1=OP.add)

            nc.sync.dma_start(yout.rearrange("(cc p) i -> p cc i", p=P), ysb[:])

    nc.compile()
    return nc


def _host_pack(x, offset, mask, weight, gamma, beta):
    """Build the 8 per-core input maps (pure layout work)."""
    in_maps = []
    wts = np.ascontiguousarray(
        weight.reshape(COUT, CIN, 9).transpose(2, 1, 0)).astype(ml_dtypes.bfloat16)
    pgrid = np.arange(P)
    ind8 = (pgrid[:, None] // 16 == np.arange(8)[None, :]).astype(np.float32)
    e8 = np.ascontiguousarray(ind8.T)
    gam2 = np.ascontiguousarray(gamma.reshape(2, P).T).astype(np.float32)
    bet2 = np.ascontiguousarray(beta.reshape(2, P).T).astype(np.float32)
    selh = np.zeros((HP, HP, P), dtype=np.float32)
    for k in range(HP):
        selh[k, k, :] = 1.0
    selh = selh.reshape(HP, HP * P)
    iotap = np.arange(P, dtype=np.float32).reshape(P, 1)
    dy = (np.arange(NT) // 3 - 1).astype(np.float32)
    dx = (np.arange(NT) % 3 - 1).astype(np.float32)
    dy7 = np.broadcast_to(dy[None, :, None], (HP, NT, W)).copy()
    bxi8 = np.broadcast_to(
        dx[None, :, None] + np.arange(W, dtype=np.float32)[None, None, :] + 8.0,
        (HP, NT, W)).copy()

    for core in range(8):
        b, h = core // 2, core % 2
        # slab[cx, i, ch] = x[b, ch, h*48-7+i, cx-8], zero outside
        slab = np.zeros((P, SLAB, CIN), dtype=ml_dtypes.bfloat16)
        r0 = h * HP - 7
        ivals = [i for i in range(SLAB) if 0 <= r0 + i < H]
        rows = [r0 + i for i in ivals]
        xt = x[b].transpose(2, 1, 0).astype(ml_dtypes.bfloat16)  # [W, H, CIN]
        slab[8:8 + W, ivals[0]:ivals[0] + len(ivals), :] = xt[:, rows, :]
        offs = np.ascontiguousarray(
            offset[b, :, h * HP:(h + 1) * HP, :].reshape(18, NPIX)).astype(np.float32)
        mk = np.ascontiguousarray(
            mask[b, :, h * HP:(h + 1) * HP, :].reshape(NT, NPIX)).astype(np.float32)

        in_maps.append({
            "slabd": slab.reshape(P, SLAB * CIN),
            "offs": offs,
            "mskd": mk,
            "wtd": wts,
            "dy7d": dy7, "bxi8d": bxi8,
            "seld": selh,
            "iotapd": iotap,
            "ind8d": ind8,
            "e8d": e8,
            "gamd": gam2,
            "betd": bet2,
        })
    return in_maps


def kernel(x, offset, mask, weight, gamma, beta):
    x = np.asarray(x, dtype=np.float32)
    offset = np.asarray(offset, dtype=np.float32)
    mask = np.asarray(mask, dtype=np.float32)
    weight = np.asarray(weight, dtype=np.float32)
    gamma = np.asarray(gamma, dtype=np.float32)
    beta = np.asarray(beta, dtype=np.float32)

    if "nc" not in _CACHED:
        _CACHED["nc"] = _build_nc()
    nc = _CACHED["nc"]

    in_maps = _host_pack(x, offset, mask, weight, gamma, beta)
    res = run_bass_kernel_spmd(nc, in_maps, core_ids=list(range(8)))
    _CACHED["last_results"] = res

    out = np.empty((B, COUT, H, W), dtype=np.float32)
    for core in range(8):
        b, h = core // 2, core % 2
        out[b, :, h * HP:(h + 1) * HP, :] = res.results[core]["y"].reshape(COUT, HP, W)
    return out


# revision 5
# speedup vs baseline: 3.3755x; 3.3755x over previous
"""DyDCNv2 (modulated deformable conv 3x3 + GroupNorm) on 8 Trainium2 cores. v4.

Every data-dependent access mechanism on TRN2 (SWDGE dma_gather ~180-440
ns/idx, ap_gather ~42 ns/idx, partition_broadcast ~190 us/call, DMA
partition-broadcast ~1.5 us/descriptor) is orders of magnitude too slow for
the 166K bilinear samples per core. v4 instead performs gather+blend as
dense PE matmuls:

  val[ch, s] = sum_j  X[row yy-7+j, :, ch]^T  @  S_j[:, s]
  S_j[cx, s] = tent(pyrel[s] - j) * tent(px8[s] - cx) * msk[s]

where tent(u) = relu(1 - |u|) is exactly the bilinear weight pair for both
corners at once, X is the image slab with 8 zero-padded columns on each
side (so out-of-image samples contribute zero like the reference's
zero-padding), and S_j is built in ONE custom DVE op per window row
(TENT_MUL_DCN: out = relu(s1 - |in0 - s0|) * in1, registered below).
The per-sample factors (pyrel, px8, msk) are computed with output rows on
partitions, then replicated across the 128 partitions by a K=1 ones-matmul
on the PE (the only full-rate replicator on this chip). The blended values
come out of PSUM in exactly the [cin, pix] layout the 9-tap conv matmuls
consume. GroupNorm statistics are reduced across the core pair sharing a
batch with a tiny AllReduce.

Sharding: core c handles batch b=c//2, row-half h=c%2 (48 of 96 rows).
"""

import os
import numpy as np
import ml_dtypes

import concourse.bass as bass
import concourse.bacc as bacc
import concourse.mybir as mybir
import concourse.tile as tile
import concourse.dve_ops as dve_ops
from concourse.dve_spec import Spec, lower as dve_lower, relu, maxx, Src0, Src1, C0, C1
from concourse.dve_uop import DveOpSpec
from concourse import library_config
from concourse.bass_utils import run_bass_kernel_spmd

P = 128
B, CIN, COUT, H, W = 4, 256, 256, 96, 96
HP = H // 2              # output rows per core
NPIX = HP * W            # 4608 output pixels per core
NT = 9                   # 3x3 taps
WIN = 14                 # source-row window per output row
SLAB = HP + WIN          # 62 slab rows resident per core
NS = NT * W              # 864 samples per output row (tap-major)
GN_GROUPS = 16
EPS = 1e-5
NG = (CIN // GN_GROUPS) * (H * W)  # elements per GN group (full image)

F32 = mybir.dt.float32
BF16 = mybir.dt.bfloat16
F16 = mybir.dt.float16
AX = mybir.AxisListType
OP = mybir.AluOpType

ITERS = int(os.environ.get("DCN_ITERS", "1"))
NOCC = os.environ.get("DCN_NOCC") == "1"
NOGMM = os.environ.get("DCN_NOGMM") == "1"
NOSJ = os.environ.get("DCN_NOSJ") == "1"
NOFREP = os.environ.get("DCN_NOFREP") == "1"
NOCONV = os.environ.get("DCN_NOCONV") == "1"

_CACHED = {}

_TENT_NAME = "TENT_MUL_DCN"


def _register_tent():
    """Register out = relu(s1 - |in0 - s0|) * in1 as a custom DVE op."""
    for op in dve_ops.OPS:
        if op.name == _TENT_NAME:
            return op
    row = dve_ops._CUSTOM_DVE_ROW_BASE + len(dve_ops.OPS)
    assert row < 0x20

    def _ref(in0, in1, s0, s1, imm2):
        a = np.maximum(in0.astype(np.float32) - s0, s0 - in0.astype(np.float32))
        return np.maximum(np.nan_to_num(s1 - a, nan=0.0), 0).astype(np.float32) * in1

    spec = Spec(
        body=relu(C1 - maxx(Src0 - C0, C0 - Src0)) * Src1,
        reference=_ref,
    )
    dve_ops._SUB_OPCODE_FOR_NAME[_TENT_NAME] = row
    shas = {}
    for ver in ("v3", "v4"):
        u = dve_lower(spec, ver=ver)
        shas[ver] = DveOpSpec(name=_TENT_NAME, opcode=row, uops=u, rd1_en=True).sha(ver)
    op = dve_ops.DveOp(_TENT_NAME, spec, subdim=False, uops_sha=shas)
    dve_ops.OPS.append(op)
    dve_ops.CUSTOM_DVE_SPECS[_TENT_NAME] = spec
    return op


TENT_OP = _register_tent()


def _build_nc():
    nc = bacc.Bacc("TRN2", target_bir_lowering=False, debug=False, num_devices=8)

    slabd = nc.dram_tensor("slabd", [P, SLAB * CIN], BF16, kind="ExternalInput")
    offs = nc.dram_tensor("offs", [18, NPIX], F32, kind="ExternalInput")
    mskd = nc.dram_tensor("mskd", [NT, NPIX], F32, kind="ExternalInput")
    wtd = nc.dram_tensor("wtd", [NT, CIN, COUT], BF16, kind="ExternalInput")
    dy7d = nc.dram_tensor("dy7d", [HP, NT, W], F32, kind="ExternalInput")
    bxi8d = nc.dram_tensor("bxi8d", [HP, NT, W], F32, kind="ExternalInput")
    seld = nc.dram_tensor("seld", [HP, HP * P], F32, kind="ExternalInput")
    iotapd = nc.dram_tensor("iotapd", [P, 1], F32, kind="ExternalInput")
    ind8d = nc.dram_tensor("ind8d", [P, 8], F32, kind="ExternalInput")
    e8d = nc.dram_tensor("e8d", [8, P], F32, kind="ExternalInput")
    gamd = nc.dram_tensor("gamd", [P, 2], F32, kind="ExternalInput")
    betd = nc.dram_tensor("betd", [P, 2], F32, kind="ExternalInput")
    yout = nc.dram_tensor("y", [COUT, NPIX], F32, kind="ExternalOutput")

    nc.gpsimd.load_library(library_config.mlp)

    with tile.TileContext(nc) as tc:
        with (
            tc.tile_pool(name="const", bufs=1) as cp,
            tc.tile_pool(name="dram", bufs=1, space="DRAM") as dp,
        ):
            # ---------------- true constants ----------------
            dy7 = cp.tile([HP, NT, W], F32, tag="dy7")
            nc.sync.dma_start(dy7[:], dy7d[:])
            bxi8 = cp.tile([HP, NT, W], F32, tag="bxi8")
            nc.sync.dma_start(bxi8[:], bxi8d[:])
            sel = cp.tile([HP, HP * P], F32, tag="sel")
            nc.sync.dma_start(sel[:], seld[:])
            iotap = cp.tile([P, 1], F32, tag="iotap")
            nc.sync.dma_start(iotap[:], iotapd[:])
            ind8 = cp.tile([P, 8], F32, tag="ind8")
            nc.sync.dma_start(ind8[:], ind8d[:])
            e8 = cp.tile([8, P], F32, tag="e8")
            nc.sync.dma_start(e8[:], e8d[:])
            gam = cp.tile([P, 2], F32, tag="gam")
            nc.sync.dma_start(gam[:], gamd[:])
            bet = cp.tile([P, 2], F32, tag="bet")
            nc.sync.dma_start(bet[:], betd[:])
            wtile = cp.tile([P, NT, 2, COUT], BF16, tag="wtile")
            nc.sync.dma_start(wtile[:], wtd.rearrange("t (kc p) o -> p t kc o", p=P))

            # persistent per-iter tensors
            slab = cp.tile([P, SLAB, CIN], BF16, tag="slab")
            cmp_ = cp.tile([HP, 3, NT, W], F32, tag="cmp")
            ysb = cp.tile([P, 2, NPIX], F32, tag="ysb")

            for _it in range(ITERS):
                nc.sync.dma_start(slab[:], slabd.rearrange("p (r c) -> p r c", c=CIN))

                # per-sample factors, output rows on partitions:
                # cmp[yy, 0, t, x] = pyrel = dy(t) + offy   (window-centered)
                # cmp[yy, 1, t, x] = px8   = x + dx(t) + offx + 8
                # cmp[yy, 2, t, x] = mask
                with tc.tile_pool(name="setup", bufs=1) as wk:
                    offw = offs.rearrange("(t two) (yy x) -> two yy t x", two=2, x=W)
                    offy = wk.tile([HP, NT, W], F32, tag="offy")
                    nc.sync.dma_start(offy[:], offw[0])
                    offx = wk.tile([HP, NT, W], F32, tag="offx")
                    nc.sync.dma_start(offx[:], offw[1])
                    nc.sync.dma_start(cmp_[:, 2],
                                      mskd.rearrange("t (yy x) -> yy t x", x=W))
                    nc.vector.tensor_tensor(cmp_[:, 0], offy[:], dy7[:], op=OP.add)
                    nc.vector.tensor_tensor(cmp_[:, 1], offx[:], bxi8[:], op=OP.add)

                # ---------------- main loop over output rows ----------------
                with (
                    tc.tile_pool(name="fr", bufs=2) as frp,
                    tc.tile_pool(name="bxp", bufs=2) as bxp,
                    tc.tile_pool(name="sj", bufs=4) as sjp,
                    tc.tile_pool(name="vs", bufs=2) as vsp,
                    tc.tile_pool(name="fp", bufs=1, space="PSUM") as fpp,
                    tc.tile_pool(name="val", bufs=1, space="PSUM") as valp,
                    tc.tile_pool(name="accp", bufs=1, space="PSUM") as accp,
                ):
                    for yy in range(HP):
                        # replicate this row's factors across all 128 partitions
                        fy16 = frp.tile([P, NS], F16, tag="fy16")
                        frep = frp.tile([P, 2, NS], F32, tag="frep")
                        if NOFREP:
                            nc.vector.memset(frep[:, :, 0:4], 0)
                            nc.vector.memset(fy16[:, 0:4], 0)
                        for f in (range(0) if NOFREP else range(3)):
                            fpsum = fpp.tile([P, 1024], F32, tag="fpsum",
                                             name="fpsum")
                            src = cmp_[:, f]
                            for lo, hi in ((0, 512), (512, NS)):
                                nc.tensor.matmul(
                                    fpsum[:, lo:hi],
                                    sel[:, yy * P:(yy + 1) * P],
                                    bass.AP(src.tensor, src.offset + lo,
                                            [src.ap[0], [1, hi - lo]]),
                                    start=True, stop=True)
                            if f == 0:
                                nc.scalar.copy(out=fy16[:], in_=fpsum[:, 0:NS])
                            else:
                                nc.scalar.copy(out=frep[:, f - 1, :],
                                               in_=fpsum[:, 0:NS])

                        # Bx[cx, s] = tent(px8[s] - cx) * msk[s]
                        bxm = bxp.tile([P, NS], F16, tag="bxm")
                        nc.vector._custom_dve(
                            TENT_OP, out=bxm[:], in0=frep[:, 0, :],
                            in1=frep[:, 1, :], s0=iotap[:], s1=1.0)

                        # gather matmuls: val[ch, s] accumulated over window rows
                        val = [valp.tile([P, 1024], F32, tag=f"val{kc}",
                                         name=f"val{kc}") for kc in range(2)]
                        for j in range(WIN):
                            sj = sjp.tile([P, NS], BF16, tag="sj")
                            if not NOSJ:
                                nc.vector._custom_dve(
                                    TENT_OP, out=sj[:], in0=fy16[:],
                                    in1=bxm[:], s0=float(j - 7), s1=1.0)
                            elif j == 0:
                                nc.vector.memset(sj[:], 0)
                            for kc in range(() if NOGMM else range(2)) if False else (range(0) if NOGMM else range(2)):
                                for lo, hi in ((0, 512), (512, NS)):
                                    nc.tensor.matmul(
                                        val[kc][:, lo:hi],
                                        slab[:, yy + j, kc * P:(kc + 1) * P],
                                        sj[:, lo:hi],
                                        start=(j == 0),
                                        stop=(j == WIN - 1),
                                    )

                        valS = vsp.tile([P, 2, NS], BF16, tag="valS")
                        if NOGMM:
                            if yy == 0:
                                nc.vector.memset(valS[:], 0)
                        else:
                            for kc in range(2):
                                nc.scalar.copy(out=valS[:, kc, :], in_=val[kc][:, 0:NS])

                        # conv: 9 taps x 2 cin-halves accumulated per cout-half
                        acc = [accp.tile([P, W], F32, tag=f"acc{cc}",
                                         name=f"acc{cc}") for cc in range(2)]
                        for cc in range(0 if NOCONV else 2):
                            for kc in range(2):
                                for t in range(NT):
                                    nc.tensor.matmul(
                                        acc[cc][:],
                                        wtile[:, t, kc, cc * P:(cc + 1) * P],
                                        valS[:, kc, t * W:(t + 1) * W],
                                        start=(kc == 0 and t == 0),
                                        stop=(kc == 1 and t == NT - 1),
                                    )
                        for cc in range(0 if NOCONV else 2):
                            nc.scalar.copy(out=ysb[:, cc, yy * W:(yy + 1) * W],
                                           in_=acc[cc][:])

                # ---------------- GroupNorm ----------------
                with tc.tile_pool(name="gnp", bufs=1, space="PSUM") as gnp, \
                     tc.tile_pool(name="gns", bufs=1) as wk:
                    st = wk.tile([P, 4], F32, tag="st")
                    sq = wk.tile([P, NPIX], F32, tag="sq")
                    for cc in range(2):
                        nc.vector.reduce_sum(st[:, 2 * cc:2 * cc + 1], ysb[:, cc, :], axis=AX.X)
                        nc.vector.tensor_tensor(sq[:], ysb[:, cc, :], ysb[:, cc, :], op=OP.mult)
                        nc.vector.reduce_sum(st[:, 2 * cc + 1:2 * cc + 2], sq[:], axis=AX.X)
                    pg = gnp.tile([8, 4], F32, tag="pg")
                    nc.tensor.matmul(pg[:], ind8[:], st[:], start=True, stop=True)
                    gsb = wk.tile([8, 4], F32, tag="gsb")
                    nc.vector.tensor_copy(gsb[:], pg[:])

                    cind = dp.tile([8, 4], F32, tag="cind")
                    cout_ = dp.tile([8, 4], F32, tag="cout")
                    nc.gpsimd.dma_start(cind[:], gsb[:])
                    if NOCC:
                        nc.sync.dma_start(cout_[:], cind[:])
                    else:
                        nc.gpsimd.collective_compute(
                            "AllReduce", OP.add,
                            replica_groups=[[0, 1], [2, 3], [4, 5], [6, 7]],
                            ins=[cind.opt()], outs=[cout_.opt()],
                        )
                    nc.sync.dma_start(gsb[:], cout_[:])

                    mu = wk.tile([8, 2], F32, tag="mu")
                    e2 = wk.tile([8, 2], F32, tag="e2")
                    nc.vector.tensor_scalar(mu[:], gsb[:, 0::2], 1.0 / NG, None, op0=OP.mult)
                    nc.vector.tensor_scalar(e2[:], gsb[:, 1::2], 1.0 / NG, None, op0=OP.mult)
                    m2t = wk.tile([8, 2], F32, tag="m2t")
                    nc.vector.tensor_tensor(m2t[:], mu[:], mu[:], op=OP.mult)
                    nc.vector.tensor_tensor(e2[:], e2[:], m2t[:], op=OP.subtract)
                    nc.vector.tensor_scalar(e2[:], e2[:], EPS, None, op0=OP.add)
                    rs = wk.tile([8, 2], F32, tag="rs")
                    nc.scalar.activation(rs[:], e2[:], mybir.ActivationFunctionType.Sqrt)
                    nc.vector.reciprocal(rs[:], rs[:])

                    pex = gnp.tile([P, 2], F32, tag="pex")
                    rsc = wk.tile([P, 2], F32, tag="rsc")
                    nc.tensor.matmul(pex[:], e8[:], rs[:], start=True, stop=True)
                    nc.vector.tensor_copy(rsc[:], pex[:])
                    pex2 = gnp.tile([P, 2], F32, tag="pex2")
                    muc = wk.tile([P, 2], F32, tag="muc")
                    nc.tensor.matmul(pex2[:], e8[:], mu[:], start=True, stop=True)
                    nc.vector.tensor_copy(muc[:], pex2[:])

                    sc = wk.tile([P, 2], F32, tag="sc")
                    nc.vector.tensor_tensor(sc[:], rsc[:], gam[:], op=OP.mult)
                    sh = wk.tile([P, 2], F32, tag="sh")
                    nc.vector.tensor_tensor(sh[:], muc[:], sc[:], op=OP.mult)
                    nc.vector.tensor_tensor(sh[:], bet[:], sh[:], op=OP.subtract)

                    for cc in range(2):
                        nc.vector.tensor_scalar(
                            ysb[:, cc, :], ysb[:, cc, :],
                            sc[:, cc:cc + 1], sh[:, cc:cc + 1],
                            op0=OP.mult, op1=OP.add)

            nc.sync.dma_start(yout.rearrange("(cc p) i -> p cc i", p=P), ysb[:])

    nc.compile()
    return nc


def _host_pack(x, offset, mask, weight, gamma, beta):
    """Build the 8 per-core input maps (pure layout work)."""
    in_maps = []
    wts = np.ascontiguousarray(
        weight.reshape(COUT, CIN, 9).transpose(2, 1, 0)).astype(ml_dtypes.bfloat16)
    pgrid = np.arange(P)
    ind8 = (pgrid[:, None] // 16 == np.arange(8)[None, :]).astype(np.float32)
    e8 = np.ascontiguousarray(ind8.T)
    gam2 = np.ascontiguousarray(gamma.reshape(2, P).T).astype(np.float32)
    bet2 = np.ascontiguousarray(beta.reshape(2, P).T).astype(np.float32)
    selh = np.zeros((HP, HP, P), dtype=np.float32)
    for k in range(HP):
        selh[k, k, :] = 1.0
    selh = selh.reshape(HP, HP * P)
    iotap = np.arange(P, dtype=np.float32).reshape(P, 1)
    dy = (np.arange(NT) // 3 - 1).astype(np.float32)
    dx = (np.arange(NT) % 3 - 1).astype(np.float32)
    dy7 = np.broadcast_to(dy[None, :, None], (HP, NT, W)).copy()
    bxi8 = np.broadcast_to(
        dx[None, :, None] + np.arange(W, dtype=np.float32)[None, None, :] + 8.0,
        (HP, NT, W)).copy()

    for core in range(8):
        b, h = core // 2, core % 2
        # slab[cx, i, ch] = x[b, ch, h*48-7+i, cx-8], zero outside
        slab = np.zeros((P, SLAB, CIN), dtype=ml_dtypes.bfloat16)
        r0 = h * HP - 7
        ivals = [i for i in range(SLAB) if 0 <= r0 + i < H]
        rows = [r0 + i for i in ivals]
        xt = x[b].transpose(2, 1, 0).astype(ml_dtypes.bfloat16)  # [W, H, CIN]
        slab[8:8 + W, ivals[0]:ivals[0] + len(ivals), :] = xt[:, rows, :]
        offs = np.ascontiguousarray(
            offset[b, :, h * HP:(h + 1) * HP, :].reshape(18, NPIX)).astype(np.float32)
        mk = np.ascontiguousarray(
            mask[b, :, h * HP:(h + 1) * HP, :].reshape(NT, NPIX)).astype(np.float32)

        in_maps.append({
            "slabd": slab.reshape(P, SLAB * CIN),
            "offs": offs,
            "mskd": mk,
            "wtd": wts,
            "dy7d": dy7, "bxi8d": bxi8,
            "seld": selh,
            "iotapd": iotap,
            "ind8d": ind8,
            "e8d": e8,
            "gamd": gam2,
            "betd": bet2,
        })
    return in_maps


def kernel(x, offset, mask, weight, gamma, beta):
    x = np.asarray(x, dtype=np.float32)
    offset = np.asarray(offset, dtype=np.float32)
    mask = np.asarray(mask, dtype=np.float32)
    weight = np.asarray(weight, dtype=np.float32)
    gamma = np.asarray(gamma, dtype=np.float32)
    beta = np.asarray(beta, dtype=np.float32)

    if "nc" not in _CACHED:
        _CACHED["nc"] = _build_nc()
    nc = _CACHED["nc"]

    in_maps = _host_pack(x, offset, mask, weight, gamma, beta)
    res = run_bass_kernel_spmd(nc, in_maps, core_ids=list(range(8)))
    _CACHED["last_results"] = res

    out = np.empty((B, COUT, H, W), dtype=np.float32)
    for core in range(8):
        b, h = core // 2, core % 2
        out[b, :, h * HP:(h + 1) * HP, :] = res.results[core]["y"].reshape(COUT, HP, W)
    return out
